# revision 41
# baseline (speedup 1.0000x reference)
"""Chamfer-style Gaussian-splat matching loss on 8 Trainium2 NeuronCores.

Sharding (data-parallel over queries M): core c handles batch c//4, query
slice c%4 (2048 queries) against the full input cloud (8192) of its batch.

Single row-oriented pass per core (v10): negsq[m,n] = 2a.b - |a|^2 - |b|^2
via K=13 f32r hi/lo-split matmuls into PSUM tiles [128m x {2048|1536}n]
(4-way row-group tiling). One ACT pass exp(T*negsq) -> bf16 SBUF tiles
serves every consumer (ACT is the per-element bottleneck engine and runs
each element exactly once):
  - col side: PE ones-matmul partition-reduction of the bf16 exp tiles,
    accumulated over all 16 m-tiles into PSUM colS[16 nblk, 512] (exact
    f32 sums of bf16 exp; host does -ln(S)/T softmin per point).
  - row side: DVE tensor_tensor-max trees (2x bf16 mode) reduce each
    [128, 64 chunk, 128] exp tile to per-chunk maxes; max8/max_index pick
    the winning 128-point chunk; its bf16 score IS exp(T*rowmax) so the
    host recovers the exact row min as -ln(score)/T. Exact in-chunk argmax
    via the DRAM window gather + f16 recompute (argmax only).
  - matched attributes: per-row indirect DMA gather; rot |dot| and L1
    group sums reduced on DVE into grouped buffers, strided-reduced once.
Engine budget per core: ACT ~133us (the wall), DVE ~110us, PE ~75us; the
three chains are decoupled (separate PSUM pools, 2-buffer exp tiles) so
they overlap instead of serializing through the PE FIFO like the previous
two-orientation design.
"""
import numpy as np

B, N, M = 2, 8192, 8192
NCORES = 8
SHARDS = 4
MLOC = M // SHARDS       # 2048
NMT = MLOC // 128        # 16
DA = 56
T_SOFT = 512.0
CLAMP_D2 = 92.0 / T_SOFT

POS_W, ROT_W, SCALE_W, OPAC_W, SH_W = 1.0, 0.5, 0.5, 0.3, 0.2

# (psum pool id, n offset, width) — pools strictly alternate, including
# across tile boundaries (even tiles run A-B-A-B-A, odd tiles B-A-B-A-B),
# and every width >=1024 so each exp instr covers the next dist-mm fill.
# Pool A holds up to 2048 (4 banks), pool B up to 1536 (3 banks).
SUBTILES_EVEN = [(0, 0, 2048), (1, 2048, 1536), (0, 3584, 2048),
                 (1, 5632, 1536), (0, 7168, 1024)]
SUBTILES_ODD = [(1, 0, 1536), (0, 1536, 2048), (1, 3584, 1536),
                (0, 5120, 2048), (1, 7168, 1024)]

_cache = {}


def _rn12(x):
    u = np.ascontiguousarray(x.astype(np.float32)).view(np.uint32)
    lsb = (u >> np.uint32(12)) & np.uint32(1)
    rounded = u + np.uint32(0x7FF) + lsb
    return (rounded & np.uint32(0xFFFFF000)).view(np.float32)


def _build_program(R=1, feat="all"):
    # feat: "dist" | "exp" | "exp+colS" | "exp+tree" | "all" — partial
    # pipelines for engine-level benchmarking; "all" is the real kernel.
    # R>1 replicates the body via a hardware loop (timing benchmarks only).
    has_exp = feat != "dist"
    has_cols = feat in ("exp+colS", "all")
    has_tree = feat in ("exp+tree", "all")
    has_pick = feat == "all"
    from contextlib import ExitStack, nullcontext
    import concourse.bass as bass
    import concourse.bacc as bacc
    import concourse.tile as tile
    from concourse import mybir

    F32 = mybir.dt.float32
    F16 = mybir.dt.float16
    BF16 = mybir.dt.bfloat16
    F32R = mybir.dt.float32r
    U32 = mybir.dt.uint32
    AX = mybir.AxisListType.X
    MAX = mybir.AluOpType.max
    ADD = mybir.AluOpType.add
    SUB = mybir.AluOpType.subtract
    MULT = mybir.AluOpType.mult
    ABSMAX = mybir.AluOpType.abs_max
    Exp = mybir.ActivationFunctionType.Exp

    nc = bacc.Bacc("TRN2", target_bir_lowering=False, debug=False)

    # ab: 4 replica blocks of [13, 2048 A | 2048 B-quarter]; group g streams
    # only chunks c with c%4==g, so each row-group replica carries N/4 B cols.
    ab_d = nc.dram_tensor("ab", [13, 4 * 4096], F32R, kind="ExternalInput").ap()
    w_d = nc.dram_tensor("w", [128, 256], F16, kind="ExternalInput").ap()
    asml_d = nc.dram_tensor("asml", [128, 4 * NMT], F32, kind="ExternalInput").ap()
    inattr_d = nc.dram_tensor("in_attr", [N, DA], F16, kind="ExternalInput").ap()
    outattr_d = nc.dram_tensor("out_attr", [128, DA * NMT], F16, kind="ExternalInput").ap()
    oneh_d = nc.dram_tensor("oneh", [128, 32], BF16, kind="ExternalInput").ap()
    # out_all: 0:16 row chunk score | 16:32 rotabs | 32:48 scale | 48:64 opac
    #          64:80 shdc | 80:96 shrest
    out_d = nc.dram_tensor("out_all", [128, 96], F32, kind="ExternalOutput").ap()
    outcs_d = nc.dram_tensor("out_cs", [128, 512], F32, kind="ExternalOutput").ap()

    with tile.TileContext(nc) as tc:
        with ExitStack() as ctx:
            const_pool = ctx.enter_context(tc.tile_pool(name="const", bufs=1))
            expo_pool = ctx.enter_context(tc.tile_pool(name="expo", bufs=3))
            tree_pool = ctx.enter_context(tc.tile_pool(name="tree", bufs=2))
            small_pool = ctx.enter_context(tc.tile_pool(name="small", bufs=6))
            psA_pool = ctx.enter_context(tc.tile_pool(name="psA", bufs=1, space="PSUM"))
            psB_pool = ctx.enter_context(tc.tile_pool(name="psB", bufs=1, space="PSUM"))
            psC_pool = ctx.enter_context(tc.tile_pool(name="psC", bufs=1, space="PSUM"))

            ab_s = const_pool.tile([128, 4096], F32R)
            # 8 half-transfers spread over the 3 DMA-capable queues so the
            # slowest queue clears in ~3 half-units instead of 2 full ones
            for _g, _h, _eng in ((0, 0, nc.sync), (0, 1, nc.gpsimd),
                                 (1, 0, nc.scalar), (1, 1, nc.sync),
                                 (2, 0, nc.gpsimd), (2, 1, nc.scalar),
                                 (3, 0, nc.sync), (3, 1, nc.gpsimd)):
                _eng.dma_start(
                    ab_s[32 * _g:32 * _g + 13, 2048 * _h:2048 * (_h + 1)],
                    ab_d[:, _g * 4096 + 2048 * _h:_g * 4096 + 2048 * (_h + 1)])
            asml_s = const_pool.tile([128, 4 * NMT], F32)
            nc.sync.dma_start(asml_s[:], asml_d[:])
            outattr_s = const_pool.tile([128, DA * NMT], F16)
            nc.sync.dma_start(outattr_s[:], outattr_d[:])
            oneh_s = const_pool.tile([128, 32], BF16)
            nc.sync.dma_start(oneh_s[:], oneh_d[:])

            warm = const_pool.tile([128, 8], F32)
            nc.vector.memset(warm[:], 0.0)
            if has_exp:
                nc.scalar.activation(warm[:], warm[:],
                                     mybir.ActivationFunctionType.Exp)

            out_all = const_pool.tile([128, 96], F32)
            # bigabs: per-tile |attr diff| blocks of 52 (scale 3|opac 1|shdc 3|shrest 45)
            bigabs = const_pool.tile([128, 832], F32)
            colS_s = const_pool.tile([128, 512], F32)
            if not has_pick:
                nc.vector.memset(out_all[:], 0.0)
            if not has_cols:
                nc.vector.memset(colS_s[:], 0.0)

            with (tc.For_i(0, R) if R > 1 else nullcontext()):
                psC = psC_pool.tile([128, 512], F32, tag="psC")
                colS_idx = [0]
                # colS matmuls wait 2 subtiles behind dist matmuls on the PE
                # queue so a pending colS never delays the next psum refill
                pending = []
                # per-tile state for the 2-deep software pipeline
                st = {}

                def emit_pending(flush=False):
                    while pending and (flush or len(pending) > 2):
                        expo_p, chunks = pending.pop(0)
                        for blk in chunks:
                            idx = colS_idx[0]
                            nc.tensor.matmul(
                                psC[0:16, :],
                                oneh_s[:, 15 - blk:31 - blk],
                                expo_p[:, blk * 512:(blk + 1) * 512],
                                start=(idx == 0), stop=(idx == 16 * NMT - 1))
                            colS_idx[0] += 1

                def emit_dist_exp(t):
                    expo = expo_pool.tile([128, N], BF16, tag="expo")
                    if t == 0 and R == 1:
                        # ramp-special: each subtile uses one replica's chunk
                        # set so exp starts as soon as that replica's DMA
                        # lands (replicas arrive in order g0, g1, g2, g3)
                        subtiles = [(0, [0, 4, 8, 12]), (1, [1, 5, 9]),
                                    (0, [2, 6, 10, 14]), (1, [3, 7, 11]),
                                    (0, [13, 15])]
                    else:
                        pat = SUBTILES_EVEN if t % 2 == 0 else SUBTILES_ODD
                        subtiles = [(pid, list(range(off // 512,
                                                     (off + width) // 512)))
                                    for (pid, off, width) in pat]
                    for (pid, chunks) in subtiles:
                        pool = psA_pool if pid == 0 else psB_pool
                        psw = 2048 if pid == 0 else 1536
                        ps = pool.tile([128, psw], F32, tag=f"ps{pid}")
                        for k, c in enumerate(chunks):
                            gb = 32 * (c % 4)
                            nc.tensor.matmul(
                                ps[:, k * 512:(k + 1) * 512],
                                ab_s[gb:gb + 13, t * 128:(t + 1) * 128],
                                ab_s[gb:gb + 13, 2048 + (c // 4) * 512:
                                     2048 + (c // 4 + 1) * 512],
                                start=True, stop=True, tile_position=(gb, 0))
                        if has_cols:
                            emit_pending()
                        if has_exp:
                            w = 512 * len(chunks)
                            step = (chunks[1] - chunks[0]
                                    if len(chunks) > 1 else 1)
                            if step == 1:
                                ov = expo[:, chunks[0] * 512:
                                          chunks[0] * 512 + w]
                            else:
                                # arithmetic chunk sequence: strided view
                                c0 = chunks[0] // step
                                s0 = chunks[0] % step
                                ov = expo[:].rearrange(
                                    "p (c s x) -> p c s x", s=step, x=512
                                )[:, c0:c0 + len(chunks), s0:s0 + 1, :]
                            nc.scalar.activation(ov, ps[:, 0:w], Exp,
                                                 scale=T_SOFT)
                        if has_cols:
                            pending.append((expo, chunks))
                    return expo

                def emit_tree_piece(t, expo, scores, c0, c1):
                    # per-chunk (64-pt) maxes for chunks [c0, c1) via tt-max tree
                    nch = c1 - c0
                    prev = expo[:, c0 * 64:c1 * 64]
                    for w in (32, 16, 8, 4, 2, 1):
                        if w == 1:
                            cur = scores[:, c0:c1]
                        else:
                            lvl = tree_pool.tile([128, 128 * w], BF16,
                                                 name=f"lvl{w}", tag=f"L{w}")
                            cur = lvl[:, 0:nch * w]
                        v = prev.rearrange("p (c x) -> p c x", x=2 * w)
                        nc.vector.tensor_tensor(
                            out=cur.rearrange("p (c x) -> p c x", x=w),
                            in0=v[:, :, 0:w], in1=v[:, :, w:2 * w], op=MAX)
                        prev = cur

                def emit_pick(t, scores):
                    top8 = small_pool.tile([128, 8], BF16, tag="top8")
                    ci = small_pool.tile([128, 8], U32, tag="ci")
                    nc.vector.max(top8[:], scores[:])
                    nc.vector.max_index(ci[:], top8[:], scores[:])
                    nc.vector.tensor_copy(out_all[:, t:t + 1], top8[:, 0:1])
                    wnd = small_pool.tile([128, 256], F16, tag="wnd")
                    nc.gpsimd.indirect_dma_start(
                        out=wnd[:], out_offset=None, in_=w_d[:],
                        in_offset=bass.IndirectOffsetOnAxis(ap=ci[:, 0:1], axis=0))
                    st[t] = {"ci": ci, "wnd": wnd}

                def emit_refine(t):
                    s = st[t]
                    wnd = s["wnd"]
                    a0 = asml_s[:, 4 * t + 0:4 * t + 1]
                    a1 = asml_s[:, 4 * t + 1:4 * t + 2]
                    a2 = asml_s[:, 4 * t + 2:4 * t + 3]
                    t1 = small_pool.tile([128, 64], F16, tag="t1")
                    t2 = small_pool.tile([128, 64], F16, tag="t2")
                    t3 = small_pool.tile([128, 64], F16, tag="t3")
                    nc.vector.tensor_scalar(out=t1[:], in0=wnd[:, 0:64],
                                            scalar1=a0, scalar2=None, op0=MULT)
                    nc.vector.tensor_scalar(out=t2[:], in0=wnd[:, 64:128],
                                            scalar1=a1, scalar2=None, op0=MULT)
                    nc.vector.tensor_scalar(out=t3[:], in0=wnd[:, 128:192],
                                            scalar1=a2, scalar2=None, op0=MULT)
                    s1 = small_pool.tile([128, 64], F16, tag="s1")
                    s2 = small_pool.tile([128, 64], F16, tag="s2")
                    negw = small_pool.tile([128, 64], F16, tag="negw")
                    nc.vector.tensor_tensor(out=s1[:], in0=t1[:], in1=t2[:], op=ADD)
                    nc.vector.tensor_tensor(out=s2[:], in0=t3[:], in1=wnd[:, 192:256], op=ADD)
                    nc.vector.tensor_tensor(out=negw[:], in0=s1[:], in1=s2[:], op=ADD)
                    wtop = small_pool.tile([128, 8], F16, tag="wtop")
                    wli = small_pool.tile([128, 8], U32, tag="wli")
                    nc.vector.max(wtop[:], negw[:])
                    nc.vector.max_index(wli[:], wtop[:], negw[:])
                    cf = small_pool.tile([128, 1], F32, tag="cf")
                    lf = small_pool.tile([128, 1], F32, tag="lf")
                    mf = small_pool.tile([128, 1], F32, tag="mf")
                    mi = small_pool.tile([128, 1], U32, tag="mi")
                    nc.vector.tensor_copy(cf[:], s["ci"][:, 0:1])
                    nc.vector.tensor_copy(lf[:], wli[:, 0:1])
                    nc.vector.tensor_scalar(out=mf[:], in0=cf[:], scalar1=64.0,
                                            scalar2=lf[:, 0:1], op0=MULT, op1=ADD)
                    nc.vector.tensor_copy(mi[:], mf[:])
                    g = small_pool.tile([128, DA], F16, tag="g")
                    nc.gpsimd.indirect_dma_start(
                        out=g[:], out_offset=None, in_=inattr_d[:],
                        in_offset=bass.IndirectOffsetOnAxis(ap=mi[:, 0:1], axis=0))
                    s["g"] = g

                def emit_attr(t):
                    g = st.pop(t)["g"]
                    oat = outattr_s[:, DA * t:DA * (t + 1)]
                    diffa = small_pool.tile([128, DA - 4], F32, tag="diffa")
                    diffb = small_pool.tile([128, DA - 4], F32, tag="diffb")
                    nc.vector.tensor_tensor(out=diffa[:], in0=oat[:, 4:DA],
                                            in1=g[:, 4:DA], op=SUB)
                    nc.vector.tensor_tensor(out=diffb[:], in0=g[:, 4:DA],
                                            in1=oat[:, 4:DA], op=SUB)
                    nc.vector.tensor_tensor(out=bigabs[:, 52 * t:52 * t + 52],
                                            in0=diffa[:], in1=diffb[:], op=MAX)
                    rotm = small_pool.tile([128, 4], F32, tag="rotm")
                    rotd = small_pool.tile([128, 1], F32, tag="rotd")
                    rotn = small_pool.tile([128, 1], F32, tag="rotn")
                    nc.vector.tensor_tensor(out=rotm[:], in0=oat[:, 0:4],
                                            in1=g[:, 0:4], op=MULT)
                    nc.vector.tensor_reduce(rotd[:], rotm[:], axis=AX, op=ADD)
                    nc.vector.tensor_scalar(out=rotn[:], in0=rotd[:], scalar1=-1.0,
                                            scalar2=None, op0=MULT)
                    nc.vector.tensor_tensor(out=out_all[:, 16 + t:17 + t],
                                            in0=rotd[:], in1=rotn[:], op=MAX)

                # DVE emission order per iteration is readiness-ordered so no
                # ready op FIFO-blocks behind a not-yet-ready one: tree piece 1
                # unblocks at exp(t,s4), piece 2 at exp(t,s5); refine(t-1) and
                # attr(t-2) are ready when emitted.
                def emit_attr_reduce(t0, t1):
                    # per-tile |attr diff| group sums for tiles [t0, t1)
                    bav = bigabs[:, 52 * t0:52 * t1].rearrange(
                        "p (t d) -> p t d", d=52)
                    for lo, hi, col in ((0, 3, 32), (3, 4, 48), (4, 7, 64),
                                        (7, 52, 80)):
                        nc.vector.tensor_reduce(
                            out_all[:, col + t0:col + t1], bav[:, :, lo:hi],
                            axis=AX, op=ADD)

                for t in range(NMT + 2):
                    if t < NMT:
                        expo = emit_dist_exp(t)
                        if has_tree:
                            scores = tree_pool.tile([128, 128], BF16,
                                                    tag="scores")
                            if t == NMT - 1:
                                # last tile: per-subtile subtrees so only the
                                # final 1024-col piece trails the last exp
                                subtiles = (SUBTILES_EVEN if t % 2 == 0
                                            else SUBTILES_ODD)
                                for (_, off, width) in subtiles:
                                    emit_tree_piece(t, expo, scores, off // 64,
                                                    (off + width) // 64)
                            else:
                                emit_tree_piece(t, expo, scores, 0, 112)
                                emit_tree_piece(t, expo, scores, 112, 128)
                            if has_pick:
                                emit_pick(t, scores)
                        if t == NMT - 1 and has_cols:
                            emit_pending(flush=True)
                            nc.vector.tensor_copy(colS_s[:], psC[:])
                    if not has_pick:
                        continue
                    if 1 <= t <= NMT:
                        emit_refine(t - 1)
                    if t == NMT:
                        # tiles 0..13 attrs are complete by now; their group
                        # sums reduce here so only tiles 14-15 trail the drain
                        emit_attr_reduce(0, NMT - 2)
                    if t >= 2:
                        emit_attr(t - 2)
                if has_pick:
                    emit_attr_reduce(NMT - 2, NMT)

            nc.sync.dma_start(out_d[:], out_all[:])
            nc.sync.dma_start(outcs_d[:], colS_s[:])

    nc.compile()
    return nc


def _build_runner():
    """Build the jitted SPMD callable ONCE (jax retrace per call is ~130ms)."""
    import jax
    from jax.sharding import Mesh, PartitionSpec
    from jax.experimental.shard_map import shard_map
    from concourse import mybir
    import concourse.bass2jax as b2j

    nc = _build_program()
    b2j.install_neuronx_cc_hook()

    partition_name = nc.partition_id_tensor.name if nc.partition_id_tensor else None
    in_names, out_names, out_avals, zero_outs = [], [], [], []
    for alloc in nc.m.functions[0].allocations:
        if not isinstance(alloc, mybir.MemoryLocationSet):
            continue
        name = alloc.memorylocations[0].name
        if alloc.kind == "ExternalInput":
            if name != partition_name:
                in_names.append(name)
        elif alloc.kind == "ExternalOutput":
            out_names.append(name)
            shape = tuple(alloc.tensor_shape)
            dtype = mybir.dt.np(alloc.dtype)
            out_avals.append(jax.core.ShapedArray(shape, dtype))
            zero_outs.append(np.zeros(shape, dtype))
    n_params = len(in_names)
    n_outs = len(out_avals)
    all_in_names = list(in_names) + list(out_names)
    if partition_name is not None:
        all_in_names.append(partition_name)
    donate = tuple(range(n_params, n_params + n_outs))

    def _body(*args):
        operands = list(args)
        if partition_name is not None:
            operands.append(b2j.partition_id_tensor())
        outs = b2j._bass_exec_p.bind(
            *operands,
            out_avals=tuple(out_avals),
            in_names=tuple(all_in_names),
            out_names=tuple(out_names),
            lowering_input_output_aliases=(),
            sim_require_finite=True,
            sim_require_nnan=True,
            nc=nc,
        )
        return tuple(outs)

    devices = jax.devices()[:NCORES]
    mesh = Mesh(np.asarray(devices), ("core",))
    in_specs = (PartitionSpec("core"),) * (n_params + n_outs)
    out_specs = (PartitionSpec("core"),) * n_outs
    sharded = jax.jit(
        shard_map(_body, mesh=mesh, in_specs=in_specs, out_specs=out_specs,
                  check_rep=False),
        donate_argnums=donate, keep_unused=True,
    )

    from jax.sharding import NamedSharding
    core_sharding = NamedSharding(mesh, PartitionSpec("core"))

    def prepare(in_maps):
        concat_in = [
            np.concatenate([np.asarray(in_maps[c][name]) for c in range(NCORES)], axis=0)
            for name in in_names
        ]
        return [jax.device_put(a, core_sharding) for a in concat_in]

    def execute(dev_in):
        concat_zeros = [np.zeros((NCORES * z.shape[0], *z.shape[1:]), z.dtype)
                        for z in zero_outs]
        out_arrs = sharded(*dev_in, *concat_zeros)
        return [
            {name: np.asarray(out_arrs[i]).reshape(NCORES, *out_avals[i].shape)[c]
             for i, name in enumerate(out_names)}
            for c in range(NCORES)
        ]

    def run(in_maps):
        return execute(prepare(in_maps))

    run.prepare = prepare
    run.execute = execute
    return run


def _prep_core_inputs(core, in_xyz, in_attr_cat, out_xyz, out_attr_cat):
    import ml_dtypes

    b = core // SHARDS
    s = core % SHARDS
    a_xyz = np.ascontiguousarray(out_xyz[b, s * MLOC:(s + 1) * MLOC]).astype(np.float32)
    b_xyz = np.ascontiguousarray(in_xyz[b]).astype(np.float32)

    twoa = (2.0 * a_xyz.astype(np.float64)).astype(np.float32)
    ah = _rn12(twoa)
    al = _rn12(twoa - ah)
    bb = b_xyz.astype(np.float32)
    bh = _rn12(bb)
    bl = _rn12(bb - bh)
    na = -(a_xyz.astype(np.float64) ** 2).sum(-1)
    nb = -(b_xyz.astype(np.float64) ** 2).sum(-1)
    nah = _rn12(na.astype(np.float32))
    nal = _rn12((na - nah.astype(np.float64)).astype(np.float32))
    nbh = _rn12(nb.astype(np.float32))
    nbl = _rn12((nb - nbh.astype(np.float64)).astype(np.float32))
    om = np.ones((MLOC,), np.float32)
    on = np.ones((N,), np.float32)
    A13 = np.stack([ah[:, 0], ah[:, 0], al[:, 0],
                    ah[:, 1], ah[:, 1], al[:, 1],
                    ah[:, 2], ah[:, 2], al[:, 2],
                    nah, nal, om, om], axis=0)
    B13 = np.stack([bh[:, 0], bl[:, 0], bh[:, 0],
                    bh[:, 1], bl[:, 1], bh[:, 1],
                    bh[:, 2], bl[:, 2], bh[:, 2],
                    on, on, nbh, nbl], axis=0)
    ab = np.empty((13, 4 * 4096), np.float32)
    for g in range(4):
        ab[:, g * 4096:g * 4096 + 2048] = A13
        for k in range(4):
            c = 4 * k + g
            ab[:, g * 4096 + 2048 + k * 512:g * 4096 + 2048 + (k + 1) * 512] = \
                B13[:, c * 512:(c + 1) * 512]

    W = np.empty((128, 256), np.float16)
    nbf = nb.astype(np.float32)
    for c in range(128):
        sl = slice(c * 64, (c + 1) * 64)
        W[c, 0:64] = 2.0 * b_xyz[sl, 0]
        W[c, 64:128] = 2.0 * b_xyz[sl, 1]
        W[c, 128:192] = 2.0 * b_xyz[sl, 2]
        W[c, 192:256] = nbf[sl]

    naf = na.astype(np.float32)
    asml = np.stack([a_xyz[:, 0], a_xyz[:, 1], a_xyz[:, 2], naf], axis=1)
    asml_tiled = np.ascontiguousarray(
        asml.reshape(NMT, 128, 4).transpose(1, 0, 2).reshape(128, NMT * 4))

    oa = out_attr_cat[b, s * MLOC:(s + 1) * MLOC]
    oa_tiled = np.ascontiguousarray(
        oa.reshape(NMT, 128, DA).transpose(1, 0, 2).reshape(128, NMT * DA))

    oneh = np.zeros((128, 32), ml_dtypes.bfloat16)
    oneh[:, 15] = 1.0

    return {
        "ab": ab,
        "w": W,
        "asml": asml_tiled,
        "in_attr": np.ascontiguousarray(in_attr_cat[b]),
        "out_attr": oa_tiled,
        "oneh": oneh,
    }


def kernel(in_xyz, in_rot, in_scale, in_opacity, in_sh_dc, in_sh_rest,
           out_xyz, out_rot, out_scale, out_opacity, out_sh_dc, out_sh_rest):
    if "run" not in _cache:
        _cache["run"] = _build_runner()
    run = _cache["run"]

    in_attr_cat = np.concatenate(
        [in_rot, in_scale, in_opacity, in_sh_dc, in_sh_rest], axis=2
    ).astype(np.float16)
    out_attr_cat = np.concatenate(
        [out_rot, out_scale, out_opacity, out_sh_dc, out_sh_rest], axis=2
    ).astype(np.float16)

    in_maps = [
        _prep_core_inputs(c, in_xyz, in_attr_cat, out_xyz, out_attr_cat)
        for c in range(NCORES)
    ]
    # Retry once: a crashed prior tenant can leave a core transiently wedged
    # (NRT_EXEC_UNIT_UNRECOVERABLE); it recovers after one failed attempt.
    outs_all = None
    last_err = None
    for _attempt in range(3):
        try:
            outs_all = run(in_maps)
            break
        except Exception as e:  # noqa: BLE001
            last_err = e
            import time as _time
            _time.sleep(3.0)
    if outs_all is None:
        raise last_err

    row_sum = rot_sum = scale_sum = opac_sum = shdc_sum = shrest_sum = col_sum = 0.0
    for b in range(B):
        S = np.zeros((16, 512), np.float64)
        for s in range(SHARDS):
            o = outs_all[b * SHARDS + s]["out_all"]
            S += outs_all[b * SHARDS + s]["out_cs"][0:16].astype(np.float64)
            score = o[:, 0:16].astype(np.float64)
            d2 = np.where(score > 0.0,
                          -np.log(np.maximum(score, 1e-300)) / T_SOFT, CLAMP_D2)
            row_sum += np.sqrt(np.clip(d2, 0.0, None)).sum()
            rot_sum += o[:, 16:32].sum()
            scale_sum += o[:, 32:48].sum()
            opac_sum += o[:, 48:64].sum()
            shdc_sum += o[:, 64:80].sum()
            shrest_sum += o[:, 80:96].sum()
        d2c = np.where(S > 0.0, -np.log(np.maximum(S, 1e-300)) / T_SOFT, CLAMP_D2)
        col_sum += np.sqrt(np.clip(d2c, 0.0, None)).sum()

    BM = B * M
    BN = B * N
    pos_loss = (row_sum / BM + col_sum / BN) / 2.0
    rot_loss = 1.0 - rot_sum / BM
    scale_loss = scale_sum / (BM * 3)
    opacity_loss = opac_sum / BM
    sh_loss = shdc_sum / (BM * 3) + shrest_sum / (BM * 45)
    total = (POS_W * pos_loss + ROT_W * rot_loss + SCALE_W * scale_loss
             + OPAC_W * opacity_loss + SH_W * sh_loss)
    return np.float32(total)


# revision 42
# speedup vs baseline: 1.0635x; 1.0635x over previous
"""Chamfer-style Gaussian-splat matching loss on 8 Trainium2 NeuronCores.

Sharding (data-parallel over queries M): core c handles batch c//4, query
slice c%4 (2048 queries) against the full input cloud (8192) of its batch.

Single row-oriented pass per core (v10): negsq[m,n] = 2a.b - |a|^2 - |b|^2
via K=13 f32r hi/lo-split matmuls into PSUM tiles [128m x {2048|1536}n]
(4-way row-group tiling). One ACT pass exp(T*negsq) -> bf16 SBUF tiles
serves every consumer (ACT is the per-element bottleneck engine and runs
each element exactly once):
  - col side: PE ones-matmul partition-reduction of the bf16 exp tiles,
    accumulated over all 16 m-tiles into PSUM colS[16 nblk, 512] (exact
    f32 sums of bf16 exp; host does -ln(S)/T softmin per point).
  - row side: DVE tensor_tensor-max trees (2x bf16 mode) reduce each
    [128, 64 chunk, 128] exp tile to per-chunk maxes; max8/max_index pick
    the winning 128-point chunk; its bf16 score IS exp(T*rowmax) so the
    host recovers the exact row min as -ln(score)/T. Exact in-chunk argmax
    via the DRAM window gather + f16 recompute (argmax only).
  - matched attributes: per-row indirect DMA gather; rot |dot| and L1
    group sums reduced on DVE into grouped buffers, strided-reduced once.
Engine budget per core: ACT ~133us (the wall), DVE ~110us, PE ~75us; the
three chains are decoupled (separate PSUM pools, 2-buffer exp tiles) so
they overlap instead of serializing through the PE FIFO like the previous
two-orientation design.
"""
import numpy as np

B, N, M = 2, 8192, 8192
NCORES = 8
SHARDS = 4
MLOC = M // SHARDS       # 2048
NMT = MLOC // 128        # 16
DA = 56
T_SOFT = 512.0
CLAMP_D2 = 92.0 / T_SOFT

POS_W, ROT_W, SCALE_W, OPAC_W, SH_W = 1.0, 0.5, 0.5, 0.3, 0.2

# (psum pool id, n offset, width) — pools strictly alternate, including
# across tile boundaries (even tiles run A-B-A-B-A, odd tiles B-A-B-A-B),
# and every width >=1024 so each exp instr covers the next dist-mm fill.
# Pool A holds up to 2048 (4 banks), pool B up to 1536 (3 banks).
SUBTILES_EVEN = [(0, 0, 2048), (1, 2048, 1536), (0, 3584, 2048),
                 (1, 5632, 1536), (0, 7168, 1024)]
SUBTILES_ODD = [(1, 0, 1536), (0, 1536, 2048), (1, 3584, 1536),
                (0, 5120, 2048), (1, 7168, 1024)]

_cache = {}


def _rn12(x):
    u = np.ascontiguousarray(x.astype(np.float32)).view(np.uint32)
    lsb = (u >> np.uint32(12)) & np.uint32(1)
    rounded = u + np.uint32(0x7FF) + lsb
    return (rounded & np.uint32(0xFFFFF000)).view(np.float32)


def _build_program(R=1, feat="all"):
    # feat: "dist" | "exp" | "exp+colS" | "exp+tree" | "all" — partial
    # pipelines for engine-level benchmarking; "all" is the real kernel.
    # R>1 replicates the body via a hardware loop (timing benchmarks only).
    has_exp = feat != "dist"
    has_cols = feat in ("exp+colS", "all")
    has_tree = feat in ("exp+tree", "all")
    has_pick = feat == "all"
    from contextlib import ExitStack, nullcontext
    import concourse.bass as bass
    import concourse.bacc as bacc
    import concourse.tile as tile
    from concourse import mybir

    F32 = mybir.dt.float32
    F16 = mybir.dt.float16
    BF16 = mybir.dt.bfloat16
    F32R = mybir.dt.float32r
    U32 = mybir.dt.uint32
    AX = mybir.AxisListType.X
    MAX = mybir.AluOpType.max
    ADD = mybir.AluOpType.add
    SUB = mybir.AluOpType.subtract
    MULT = mybir.AluOpType.mult
    ABSMAX = mybir.AluOpType.abs_max
    Exp = mybir.ActivationFunctionType.Exp

    nc = bacc.Bacc("TRN2", target_bir_lowering=False, debug=False)

    # ab: 4 replica blocks of [13, 2048 A | 2048 B-quarter]; group g streams
    # only chunks c with c%4==g, so each row-group replica carries N/4 B cols.
    ab_d = nc.dram_tensor("ab", [13, 4 * 4096], F32R, kind="ExternalInput").ap()
    w_d = nc.dram_tensor("w", [128, 256], F16, kind="ExternalInput").ap()
    asml_d = nc.dram_tensor("asml", [128, 4 * NMT], F32, kind="ExternalInput").ap()
    inattr_d = nc.dram_tensor("in_attr", [N, DA], F16, kind="ExternalInput").ap()
    outattr_d = nc.dram_tensor("out_attr", [128, DA * NMT], F16, kind="ExternalInput").ap()
    oneh_d = nc.dram_tensor("oneh", [128, 32], BF16, kind="ExternalInput").ap()
    # out_all: 0:16 row chunk score | 16:32 rotabs | 32:48 scale | 48:64 opac
    #          64:80 shdc | 80:96 shrest
    out_d = nc.dram_tensor("out_all", [128, 96], F32, kind="ExternalOutput").ap()
    outcs_d = nc.dram_tensor("out_cs", [128, 512], F32, kind="ExternalOutput").ap()

    with tile.TileContext(nc) as tc:
        with ExitStack() as ctx:
            const_pool = ctx.enter_context(tc.tile_pool(name="const", bufs=1))
            expo_pool = ctx.enter_context(tc.tile_pool(name="expo", bufs=3))
            tree_pool = ctx.enter_context(tc.tile_pool(name="tree", bufs=2))
            small_pool = ctx.enter_context(tc.tile_pool(name="small", bufs=6))
            psA_pool = ctx.enter_context(tc.tile_pool(name="psA", bufs=1, space="PSUM"))
            psB_pool = ctx.enter_context(tc.tile_pool(name="psB", bufs=1, space="PSUM"))
            psC_pool = ctx.enter_context(tc.tile_pool(name="psC", bufs=1, space="PSUM"))

            ab_s = const_pool.tile([128, 4096], F32R)
            # 8 half-transfers spread over the 3 DMA-capable queues so the
            # slowest queue clears in ~3 half-units instead of 2 full ones
            for _g, _h, _eng in ((0, 0, nc.sync), (0, 1, nc.gpsimd),
                                 (1, 0, nc.scalar), (1, 1, nc.sync),
                                 (2, 0, nc.gpsimd), (2, 1, nc.scalar),
                                 (3, 0, nc.sync), (3, 1, nc.gpsimd)):
                _eng.dma_start(
                    ab_s[32 * _g:32 * _g + 13, 2048 * _h:2048 * (_h + 1)],
                    ab_d[:, _g * 4096 + 2048 * _h:_g * 4096 + 2048 * (_h + 1)])
            asml_s = const_pool.tile([128, 4 * NMT], F32)
            nc.sync.dma_start(asml_s[:], asml_d[:])
            outattr_s = const_pool.tile([128, DA * NMT], F16)
            nc.sync.dma_start(outattr_s[:], outattr_d[:])
            oneh_s = const_pool.tile([128, 32], BF16)
            nc.sync.dma_start(oneh_s[:], oneh_d[:])

            warm = const_pool.tile([128, 8], F32)
            nc.vector.memset(warm[:], 0.0)
            if has_exp:
                nc.scalar.activation(warm[:], warm[:],
                                     mybir.ActivationFunctionType.Exp)

            out_all = const_pool.tile([128, 96], F32)
            # bigabs: per-tile |attr diff| blocks of 52 (scale 3|opac 1|shdc 3|shrest 45)
            bigabs = const_pool.tile([128, 832], F32)
            colS_s = const_pool.tile([128, 512], F32)
            if not has_pick:
                nc.vector.memset(out_all[:], 0.0)
            if not has_cols:
                nc.vector.memset(colS_s[:], 0.0)

            with (tc.For_i(0, R) if R > 1 else nullcontext()):
                psC = psC_pool.tile([128, 512], F32, tag="psC")
                colS_idx = [0]
                # colS matmuls wait 2 subtiles behind dist matmuls on the PE
                # queue so a pending colS never delays the next psum refill
                pending = []
                # per-tile state for the 2-deep software pipeline
                st = {}

                def emit_pending(flush=False):
                    while pending and (flush or len(pending) > 2):
                        expo_p, chunks = pending.pop(0)
                        for blk in chunks:
                            idx = colS_idx[0]
                            nc.tensor.matmul(
                                psC[0:16, :],
                                oneh_s[:, 15 - blk:31 - blk],
                                expo_p[:, blk * 512:(blk + 1) * 512],
                                start=(idx == 0), stop=(idx == 16 * NMT - 1))
                            colS_idx[0] += 1

                def emit_dist_exp(t):
                    expo = expo_pool.tile([128, N], BF16, tag="expo")
                    if t == 0 and R == 1:
                        # ramp-special: each subtile uses one replica's chunk
                        # set so exp starts as soon as that replica's DMA
                        # lands (replicas arrive in order g0, g1, g2, g3)
                        subtiles = [(0, [0, 4, 8, 12]), (1, [1, 5, 9]),
                                    (0, [2, 6, 10, 14]), (1, [3, 7, 11]),
                                    (0, [13, 15])]
                    else:
                        pat = SUBTILES_EVEN if t % 2 == 0 else SUBTILES_ODD
                        subtiles = [(pid, list(range(off // 512,
                                                     (off + width) // 512)))
                                    for (pid, off, width) in pat]
                    for (pid, chunks) in subtiles:
                        pool = psA_pool if pid == 0 else psB_pool
                        psw = 2048 if pid == 0 else 1536
                        ps = pool.tile([128, psw], F32, tag=f"ps{pid}")
                        for k, c in enumerate(chunks):
                            gb = 32 * (c % 4)
                            nc.tensor.matmul(
                                ps[:, k * 512:(k + 1) * 512],
                                ab_s[gb:gb + 13, t * 128:(t + 1) * 128],
                                ab_s[gb:gb + 13, 2048 + (c // 4) * 512:
                                     2048 + (c // 4 + 1) * 512],
                                start=True, stop=True, tile_position=(gb, 0))
                        if has_cols:
                            emit_pending()
                        if has_exp:
                            w = 512 * len(chunks)
                            step = (chunks[1] - chunks[0]
                                    if len(chunks) > 1 else 1)
                            if step == 1:
                                ov = expo[:, chunks[0] * 512:
                                          chunks[0] * 512 + w]
                            else:
                                # arithmetic chunk sequence: strided view
                                c0 = chunks[0] // step
                                s0 = chunks[0] % step
                                ov = expo[:].rearrange(
                                    "p (c s x) -> p c s x", s=step, x=512
                                )[:, c0:c0 + len(chunks), s0:s0 + 1, :]
                            nc.scalar.activation(ov, ps[:, 0:w], Exp,
                                                 scale=T_SOFT)
                        if has_cols:
                            pending.append((expo, chunks))
                    return expo

                def emit_tree_piece(t, expo, scores, c0, c1):
                    # per-chunk (64-pt) maxes for chunks [c0, c1) via tt-max tree
                    nch = c1 - c0
                    prev = expo[:, c0 * 64:c1 * 64]
                    for w in (32, 16, 8, 4, 2, 1):
                        if w == 1:
                            cur = scores[:, c0:c1]
                        else:
                            lvl = tree_pool.tile([128, 128 * w], BF16,
                                                 name=f"lvl{w}", tag=f"L{w}")
                            cur = lvl[:, 0:nch * w]
                        v = prev.rearrange("p (c x) -> p c x", x=2 * w)
                        nc.vector.tensor_tensor(
                            out=cur.rearrange("p (c x) -> p c x", x=w),
                            in0=v[:, :, 0:w], in1=v[:, :, w:2 * w], op=MAX)
                        prev = cur

                def emit_pick(t, scores):
                    top8 = small_pool.tile([128, 8], BF16, tag="top8")
                    ci = small_pool.tile([128, 8], U32, tag="ci")
                    nc.vector.max(top8[:], scores[:])
                    nc.vector.max_index(ci[:], top8[:], scores[:])
                    nc.vector.tensor_copy(out_all[:, t:t + 1], top8[:, 0:1])
                    wnd = small_pool.tile([128, 256], F16, tag="wnd")
                    nc.gpsimd.indirect_dma_start(
                        out=wnd[:], out_offset=None, in_=w_d[:],
                        in_offset=bass.IndirectOffsetOnAxis(ap=ci[:, 0:1], axis=0))
                    st[t] = {"ci": ci, "wnd": wnd}

                def emit_refine(t):
                    s = st[t]
                    wnd = s["wnd"]
                    a0 = asml_s[:, 4 * t + 0:4 * t + 1]
                    a1 = asml_s[:, 4 * t + 1:4 * t + 2]
                    a2 = asml_s[:, 4 * t + 2:4 * t + 3]
                    t1 = small_pool.tile([128, 64], F16, tag="t1")
                    t2 = small_pool.tile([128, 64], F16, tag="t2")
                    t3 = small_pool.tile([128, 64], F16, tag="t3")
                    nc.vector.tensor_scalar(out=t1[:], in0=wnd[:, 0:64],
                                            scalar1=a0, scalar2=None, op0=MULT)
                    nc.vector.tensor_scalar(out=t2[:], in0=wnd[:, 64:128],
                                            scalar1=a1, scalar2=None, op0=MULT)
                    nc.vector.tensor_scalar(out=t3[:], in0=wnd[:, 128:192],
                                            scalar1=a2, scalar2=None, op0=MULT)
                    s1 = small_pool.tile([128, 64], F16, tag="s1")
                    s2 = small_pool.tile([128, 64], F16, tag="s2")
                    negw = small_pool.tile([128, 64], F16, tag="negw")
                    nc.vector.tensor_tensor(out=s1[:], in0=t1[:], in1=t2[:], op=ADD)
                    nc.vector.tensor_tensor(out=s2[:], in0=t3[:], in1=wnd[:, 192:256], op=ADD)
                    nc.vector.tensor_tensor(out=negw[:], in0=s1[:], in1=s2[:], op=ADD)
                    wtop = small_pool.tile([128, 8], F16, tag="wtop")
                    wli = small_pool.tile([128, 8], U32, tag="wli")
                    nc.vector.max(wtop[:], negw[:])
                    nc.vector.max_index(wli[:], wtop[:], negw[:])
                    cf = small_pool.tile([128, 1], F32, tag="cf")
                    lf = small_pool.tile([128, 1], F32, tag="lf")
                    mf = small_pool.tile([128, 1], F32, tag="mf")
                    mi = small_pool.tile([128, 1], U32, tag="mi")
                    nc.vector.tensor_copy(cf[:], s["ci"][:, 0:1])
                    nc.vector.tensor_copy(lf[:], wli[:, 0:1])
                    nc.vector.tensor_scalar(out=mf[:], in0=cf[:], scalar1=64.0,
                                            scalar2=lf[:, 0:1], op0=MULT, op1=ADD)
                    nc.vector.tensor_copy(mi[:], mf[:])
                    g = small_pool.tile([128, DA], F16, tag="g")
                    nc.gpsimd.indirect_dma_start(
                        out=g[:], out_offset=None, in_=inattr_d[:],
                        in_offset=bass.IndirectOffsetOnAxis(ap=mi[:, 0:1], axis=0))
                    s["g"] = g

                def emit_attr(t):
                    g = st.pop(t)["g"]
                    oat = outattr_s[:, DA * t:DA * (t + 1)]
                    diffa = small_pool.tile([128, DA - 4], F32, tag="diffa")
                    diffb = small_pool.tile([128, DA - 4], F32, tag="diffb")
                    nc.vector.tensor_tensor(out=diffa[:], in0=oat[:, 4:DA],
                                            in1=g[:, 4:DA], op=SUB)
                    nc.vector.tensor_tensor(out=diffb[:], in0=g[:, 4:DA],
                                            in1=oat[:, 4:DA], op=SUB)
                    nc.vector.tensor_tensor(out=bigabs[:, 52 * t:52 * t + 52],
                                            in0=diffa[:], in1=diffb[:], op=MAX)
                    rotm = small_pool.tile([128, 4], F32, tag="rotm")
                    rotd = small_pool.tile([128, 1], F32, tag="rotd")
                    rotn = small_pool.tile([128, 1], F32, tag="rotn")
                    nc.vector.tensor_tensor(out=rotm[:], in0=oat[:, 0:4],
                                            in1=g[:, 0:4], op=MULT)
                    nc.vector.tensor_reduce(rotd[:], rotm[:], axis=AX, op=ADD)
                    nc.vector.tensor_scalar(out=rotn[:], in0=rotd[:], scalar1=-1.0,
                                            scalar2=None, op0=MULT)
                    nc.vector.tensor_tensor(out=out_all[:, 16 + t:17 + t],
                                            in0=rotd[:], in1=rotn[:], op=MAX)

                # DVE emission order per iteration is readiness-ordered so no
                # ready op FIFO-blocks behind a not-yet-ready one: tree piece 1
                # unblocks at exp(t,s4), piece 2 at exp(t,s5); refine(t-1) and
                # attr(t-2) are ready when emitted.
                def emit_attr_reduce(t0, t1):
                    # per-tile |attr diff| group sums for tiles [t0, t1)
                    bav = bigabs[:, 52 * t0:52 * t1].rearrange(
                        "p (t d) -> p t d", d=52)
                    for lo, hi, col in ((0, 3, 32), (3, 4, 48), (4, 7, 64),
                                        (7, 52, 80)):
                        nc.vector.tensor_reduce(
                            out_all[:, col + t0:col + t1], bav[:, :, lo:hi],
                            axis=AX, op=ADD)

                for t in range(NMT + 2):
                    if t < NMT:
                        expo = emit_dist_exp(t)
                        if has_tree:
                            scores = tree_pool.tile([128, 128], BF16,
                                                    tag="scores")
                            if t == NMT - 1:
                                # last tile: per-subtile subtrees so only the
                                # final 1024-col piece trails the last exp
                                subtiles = (SUBTILES_EVEN if t % 2 == 0
                                            else SUBTILES_ODD)
                                for (_, off, width) in subtiles:
                                    emit_tree_piece(t, expo, scores, off // 64,
                                                    (off + width) // 64)
                            else:
                                emit_tree_piece(t, expo, scores, 0, 112)
                                emit_tree_piece(t, expo, scores, 112, 128)
                            if has_pick:
                                emit_pick(t, scores)
                        if t == NMT - 1 and has_cols:
                            emit_pending(flush=True)
                            nc.vector.tensor_copy(colS_s[:], psC[:])
                    if not has_pick:
                        continue
                    if 1 <= t <= NMT:
                        emit_refine(t - 1)
                    if t >= 2:
                        emit_attr(t - 2)
                    if t == NMT - 1:
                        # tiles 0..13 attrs are complete once attr(13) above
                        # ran; reducing their group sums here keeps only
                        # tiles 14-15 in the drain tail
                        emit_attr_reduce(0, NMT - 2)
                if has_pick:
                    emit_attr_reduce(NMT - 2, NMT)

            nc.sync.dma_start(out_d[:], out_all[:])
            nc.sync.dma_start(outcs_d[:], colS_s[:])

    nc.compile()
    return nc


def _build_runner():
    """Build the jitted SPMD callable ONCE (jax retrace per call is ~130ms)."""
    import jax
    from jax.sharding import Mesh, PartitionSpec
    from jax.experimental.shard_map import shard_map
    from concourse import mybir
    import concourse.bass2jax as b2j

    nc = _build_program()
    b2j.install_neuronx_cc_hook()

    partition_name = nc.partition_id_tensor.name if nc.partition_id_tensor else None
    in_names, out_names, out_avals, zero_outs = [], [], [], []
    for alloc in nc.m.functions[0].allocations:
        if not isinstance(alloc, mybir.MemoryLocationSet):
            continue
        name = alloc.memorylocations[0].name
        if alloc.kind == "ExternalInput":
            if name != partition_name:
                in_names.append(name)
        elif alloc.kind == "ExternalOutput":
            out_names.append(name)
            shape = tuple(alloc.tensor_shape)
            dtype = mybir.dt.np(alloc.dtype)
            out_avals.append(jax.core.ShapedArray(shape, dtype))
            zero_outs.append(np.zeros(shape, dtype))
    n_params = len(in_names)
    n_outs = len(out_avals)
    all_in_names = list(in_names) + list(out_names)
    if partition_name is not None:
        all_in_names.append(partition_name)
    donate = tuple(range(n_params, n_params + n_outs))

    def _body(*args):
        operands = list(args)
        if partition_name is not None:
            operands.append(b2j.partition_id_tensor())
        outs = b2j._bass_exec_p.bind(
            *operands,
            out_avals=tuple(out_avals),
            in_names=tuple(all_in_names),
            out_names=tuple(out_names),
            lowering_input_output_aliases=(),
            sim_require_finite=True,
            sim_require_nnan=True,
            nc=nc,
        )
        return tuple(outs)

    devices = jax.devices()[:NCORES]
    mesh = Mesh(np.asarray(devices), ("core",))
    in_specs = (PartitionSpec("core"),) * (n_params + n_outs)
    out_specs = (PartitionSpec("core"),) * n_outs
    sharded = jax.jit(
        shard_map(_body, mesh=mesh, in_specs=in_specs, out_specs=out_specs,
                  check_rep=False),
        donate_argnums=donate, keep_unused=True,
    )

    from jax.sharding import NamedSharding
    core_sharding = NamedSharding(mesh, PartitionSpec("core"))

    def prepare(in_maps):
        concat_in = [
            np.concatenate([np.asarray(in_maps[c][name]) for c in range(NCORES)], axis=0)
            for name in in_names
        ]
        return [jax.device_put(a, core_sharding) for a in concat_in]

    def execute(dev_in):
        concat_zeros = [np.zeros((NCORES * z.shape[0], *z.shape[1:]), z.dtype)
                        for z in zero_outs]
        out_arrs = sharded(*dev_in, *concat_zeros)
        return [
            {name: np.asarray(out_arrs[i]).reshape(NCORES, *out_avals[i].shape)[c]
             for i, name in enumerate(out_names)}
            for c in range(NCORES)
        ]

    def run(in_maps):
        return execute(prepare(in_maps))

    run.prepare = prepare
    run.execute = execute
    return run


def _prep_core_inputs(core, in_xyz, in_attr_cat, out_xyz, out_attr_cat):
    import ml_dtypes

    b = core // SHARDS
    s = core % SHARDS
    a_xyz = np.ascontiguousarray(out_xyz[b, s * MLOC:(s + 1) * MLOC]).astype(np.float32)
    b_xyz = np.ascontiguousarray(in_xyz[b]).astype(np.float32)

    twoa = (2.0 * a_xyz.astype(np.float64)).astype(np.float32)
    ah = _rn12(twoa)
    al = _rn12(twoa - ah)
    bb = b_xyz.astype(np.float32)
    bh = _rn12(bb)
    bl = _rn12(bb - bh)
    na = -(a_xyz.astype(np.float64) ** 2).sum(-1)
    nb = -(b_xyz.astype(np.float64) ** 2).sum(-1)
    nah = _rn12(na.astype(np.float32))
    nal = _rn12((na - nah.astype(np.float64)).astype(np.float32))
    nbh = _rn12(nb.astype(np.float32))
    nbl = _rn12((nb - nbh.astype(np.float64)).astype(np.float32))
    om = np.ones((MLOC,), np.float32)
    on = np.ones((N,), np.float32)
    A13 = np.stack([ah[:, 0], ah[:, 0], al[:, 0],
                    ah[:, 1], ah[:, 1], al[:, 1],
                    ah[:, 2], ah[:, 2], al[:, 2],
                    nah, nal, om, om], axis=0)
    B13 = np.stack([bh[:, 0], bl[:, 0], bh[:, 0],
                    bh[:, 1], bl[:, 1], bh[:, 1],
                    bh[:, 2], bl[:, 2], bh[:, 2],
                    on, on, nbh, nbl], axis=0)
    ab = np.empty((13, 4 * 4096), np.float32)
    for g in range(4):
        ab[:, g * 4096:g * 4096 + 2048] = A13
        for k in range(4):
            c = 4 * k + g
            ab[:, g * 4096 + 2048 + k * 512:g * 4096 + 2048 + (k + 1) * 512] = \
                B13[:, c * 512:(c + 1) * 512]

    W = np.empty((128, 256), np.float16)
    nbf = nb.astype(np.float32)
    for c in range(128):
        sl = slice(c * 64, (c + 1) * 64)
        W[c, 0:64] = 2.0 * b_xyz[sl, 0]
        W[c, 64:128] = 2.0 * b_xyz[sl, 1]
        W[c, 128:192] = 2.0 * b_xyz[sl, 2]
        W[c, 192:256] = nbf[sl]

    naf = na.astype(np.float32)
    asml = np.stack([a_xyz[:, 0], a_xyz[:, 1], a_xyz[:, 2], naf], axis=1)
    asml_tiled = np.ascontiguousarray(
        asml.reshape(NMT, 128, 4).transpose(1, 0, 2).reshape(128, NMT * 4))

    oa = out_attr_cat[b, s * MLOC:(s + 1) * MLOC]
    oa_tiled = np.ascontiguousarray(
        oa.reshape(NMT, 128, DA).transpose(1, 0, 2).reshape(128, NMT * DA))

    oneh = np.zeros((128, 32), ml_dtypes.bfloat16)
    oneh[:, 15] = 1.0

    return {
        "ab": ab,
        "w": W,
        "asml": asml_tiled,
        "in_attr": np.ascontiguousarray(in_attr_cat[b]),
        "out_attr": oa_tiled,
        "oneh": oneh,
    }


def kernel(in_xyz, in_rot, in_scale, in_opacity, in_sh_dc, in_sh_rest,
           out_xyz, out_rot, out_scale, out_opacity, out_sh_dc, out_sh_rest):
    if "run" not in _cache:
        _cache["run"] = _build_runner()
    run = _cache["run"]

    in_attr_cat = np.concatenate(
        [in_rot, in_scale, in_opacity, in_sh_dc, in_sh_rest], axis=2
    ).astype(np.float16)
    out_attr_cat = np.concatenate(
        [out_rot, out_scale, out_opacity, out_sh_dc, out_sh_rest], axis=2
    ).astype(np.float16)

    in_maps = [
        _prep_core_inputs(c, in_xyz, in_attr_cat, out_xyz, out_attr_cat)
        for c in range(NCORES)
    ]
    # Retry once: a crashed prior tenant can leave a core transiently wedged
    # (NRT_EXEC_UNIT_UNRECOVERABLE); it recovers after one failed attempt.
    outs_all = None
    last_err = None
    for _attempt in range(3):
        try:
            outs_all = run(in_maps)
            break
        except Exception as e:  # noqa: BLE001
            last_err = e
            import time as _time
            _time.sleep(3.0)
    if outs_all is None:
        raise last_err

    row_sum = rot_sum = scale_sum = opac_sum = shdc_sum = shrest_sum = col_sum = 0.0
    for b in range(B):
        S = np.zeros((16, 512), np.float64)
        for s in range(SHARDS):
            o = outs_all[b * SHARDS + s]["out_all"]
            S += outs_all[b * SHARDS + s]["out_cs"][0:16].astype(np.float64)
            score = o[:, 0:16].astype(np.float64)
            d2 = np.where(score > 0.0,
                          -np.log(np.maximum(score, 1e-300)) / T_SOFT, CLAMP_D2)
            row_sum += np.sqrt(np.clip(d2, 0.0, None)).sum()
            rot_sum += o[:, 16:32].sum()
            scale_sum += o[:, 32:48].sum()
            opac_sum += o[:, 48:64].sum()
            shdc_sum += o[:, 64:80].sum()
            shrest_sum += o[:, 80:96].sum()
        d2c = np.where(S > 0.0, -np.log(np.maximum(S, 1e-300)) / T_SOFT, CLAMP_D2)
        col_sum += np.sqrt(np.clip(d2c, 0.0, None)).sum()

    BM = B * M
    BN = B * N
    pos_loss = (row_sum / BM + col_sum / BN) / 2.0
    rot_loss = 1.0 - rot_sum / BM
    scale_loss = scale_sum / (BM * 3)
    opacity_loss = opac_sum / BM
    sh_loss = shdc_sum / (BM * 3) + shrest_sum / (BM * 45)
    total = (POS_W * pos_loss + ROT_W * rot_loss + SCALE_W * scale_loss
             + OPAC_W * opacity_loss + SH_W * sh_loss)
    return np.float32(total)


# revision 43
# speedup vs baseline: 1.0800x; 1.0155x over previous
"""Chamfer-style Gaussian-splat matching loss on 8 Trainium2 NeuronCores.

Sharding (data-parallel over queries M): core c handles batch c//4, query
slice c%4 (2048 queries) against the full input cloud (8192) of its batch.

Single row-oriented pass per core (v10): negsq[m,n] = 2a.b - |a|^2 - |b|^2
via K=13 f32r hi/lo-split matmuls into PSUM tiles [128m x {2048|1536}n]
(4-way row-group tiling). One ACT pass exp(T*negsq) -> bf16 SBUF tiles
serves every consumer (ACT is the per-element bottleneck engine and runs
each element exactly once):
  - col side: PE ones-matmul partition-reduction of the bf16 exp tiles,
    accumulated over all 16 m-tiles into PSUM colS[16 nblk, 512] (exact
    f32 sums of bf16 exp; host does -ln(S)/T softmin per point).
  - row side: DVE tensor_tensor-max trees (2x bf16 mode) reduce each
    [128, 64 chunk, 128] exp tile to per-chunk maxes; max8/max_index pick
    the winning 128-point chunk; its bf16 score IS exp(T*rowmax) so the
    host recovers the exact row min as -ln(score)/T. Exact in-chunk argmax
    via the DRAM window gather + f16 recompute (argmax only).
  - matched attributes: per-row indirect DMA gather; rot |dot| and L1
    group sums reduced on DVE into grouped buffers, strided-reduced once.
Engine budget per core: ACT ~133us (the wall), DVE ~110us, PE ~75us; the
three chains are decoupled (separate PSUM pools, 2-buffer exp tiles) so
they overlap instead of serializing through the PE FIFO like the previous
two-orientation design.
"""
import numpy as np

B, N, M = 2, 8192, 8192
NCORES = 8
SHARDS = 4
MLOC = M // SHARDS       # 2048
NMT = MLOC // 128        # 16
DA = 56
T_SOFT = 512.0
CLAMP_D2 = 92.0 / T_SOFT

POS_W, ROT_W, SCALE_W, OPAC_W, SH_W = 1.0, 0.5, 0.5, 0.3, 0.2

# (psum pool id, n offset, width) — pools strictly alternate, including
# across tile boundaries (even tiles run A-B-A-B-A, odd tiles B-A-B-A-B),
# and every width >=1024 so each exp instr covers the next dist-mm fill.
# Pool A holds up to 2048 (4 banks), pool B up to 1536 (3 banks).
SUBTILES_EVEN = [(0, 0, 2048), (1, 2048, 1536), (0, 3584, 2048),
                 (1, 5632, 1536), (0, 7168, 1024)]
SUBTILES_ODD = [(1, 0, 1536), (0, 1536, 2048), (1, 3584, 1536),
                (0, 5120, 2048), (1, 7168, 1024)]

_cache = {}


def _rn12(x):
    u = np.ascontiguousarray(x.astype(np.float32)).view(np.uint32)
    lsb = (u >> np.uint32(12)) & np.uint32(1)
    rounded = u + np.uint32(0x7FF) + lsb
    return (rounded & np.uint32(0xFFFFF000)).view(np.float32)


def _build_program(R=1, feat="all"):
    # feat: "dist" | "exp" | "exp+colS" | "exp+tree" | "all" — partial
    # pipelines for engine-level benchmarking; "all" is the real kernel.
    # R>1 replicates the body via a hardware loop (timing benchmarks only).
    has_exp = feat != "dist"
    has_cols = feat in ("exp+colS", "all")
    has_tree = feat in ("exp+tree", "all")
    has_pick = feat == "all"
    from contextlib import ExitStack, nullcontext
    import concourse.bass as bass
    import concourse.bacc as bacc
    import concourse.tile as tile
    from concourse import mybir

    F32 = mybir.dt.float32
    F16 = mybir.dt.float16
    BF16 = mybir.dt.bfloat16
    F32R = mybir.dt.float32r
    U32 = mybir.dt.uint32
    AX = mybir.AxisListType.X
    MAX = mybir.AluOpType.max
    ADD = mybir.AluOpType.add
    SUB = mybir.AluOpType.subtract
    MULT = mybir.AluOpType.mult
    ABSMAX = mybir.AluOpType.abs_max
    Exp = mybir.ActivationFunctionType.Exp

    nc = bacc.Bacc("TRN2", target_bir_lowering=False, debug=False)

    # ab: 4 replica blocks of [13, 2048 A | 2048 B-quarter]; group g streams
    # only chunks c with c%4==g, so each row-group replica carries N/4 B cols.
    ab_d = nc.dram_tensor("ab", [13, 4 * 4096], F32R, kind="ExternalInput").ap()
    w_d = nc.dram_tensor("w", [128, 256], F16, kind="ExternalInput").ap()
    asml_d = nc.dram_tensor("asml", [128, 4 * NMT], F32, kind="ExternalInput").ap()
    inattr_d = nc.dram_tensor("in_attr", [N, DA], F16, kind="ExternalInput").ap()
    outattr_d = nc.dram_tensor("out_attr", [128, DA * NMT], F16, kind="ExternalInput").ap()
    oneh_d = nc.dram_tensor("oneh", [128, 32], BF16, kind="ExternalInput").ap()
    # out_all: 0:16 row chunk score | 16:32 rotabs | 32:48 scale | 48:64 opac
    #          64:80 shdc | 80:96 shrest
    out_d = nc.dram_tensor("out_all", [128, 96], F32, kind="ExternalOutput").ap()
    outcs_d = nc.dram_tensor("out_cs", [128, 512], F32, kind="ExternalOutput").ap()

    with tile.TileContext(nc) as tc:
        with ExitStack() as ctx:
            const_pool = ctx.enter_context(tc.tile_pool(name="const", bufs=1))
            expo_pool = ctx.enter_context(tc.tile_pool(name="expo", bufs=3))
            tree_pool = ctx.enter_context(tc.tile_pool(name="tree", bufs=2))
            small_pool = ctx.enter_context(tc.tile_pool(name="small", bufs=6))
            psA_pool = ctx.enter_context(tc.tile_pool(name="psA", bufs=1, space="PSUM"))
            psB_pool = ctx.enter_context(tc.tile_pool(name="psB", bufs=1, space="PSUM"))
            psC_pool = ctx.enter_context(tc.tile_pool(name="psC", bufs=1, space="PSUM"))

            ab_s = const_pool.tile([128, 4096], F32R)
            # 8 half-transfers spread over the 3 DMA-capable queues so the
            # slowest queue clears in ~3 half-units instead of 2 full ones
            for _g, _h, _eng in ((0, 0, nc.sync), (0, 1, nc.gpsimd),
                                 (1, 0, nc.scalar), (1, 1, nc.sync),
                                 (2, 0, nc.gpsimd), (2, 1, nc.scalar),
                                 (3, 0, nc.sync), (3, 1, nc.gpsimd)):
                _eng.dma_start(
                    ab_s[32 * _g:32 * _g + 13, 2048 * _h:2048 * (_h + 1)],
                    ab_d[:, _g * 4096 + 2048 * _h:_g * 4096 + 2048 * (_h + 1)])
            asml_s = const_pool.tile([128, 4 * NMT], F32)
            nc.sync.dma_start(asml_s[:], asml_d[:])
            outattr_s = const_pool.tile([128, DA * NMT], F16)
            nc.sync.dma_start(outattr_s[:], outattr_d[:])
            oneh_s = const_pool.tile([128, 32], BF16)
            nc.sync.dma_start(oneh_s[:], oneh_d[:])

            warm = const_pool.tile([128, 8], F32)
            nc.vector.memset(warm[:], 0.0)
            if has_exp:
                nc.scalar.activation(warm[:], warm[:],
                                     mybir.ActivationFunctionType.Exp)

            out_all = const_pool.tile([128, 96], F32)
            # bigabs: per-tile |attr diff| blocks of 52 (scale 3|opac 1|shdc 3|shrest 45)
            bigabs = const_pool.tile([128, 832], F32)
            colS_s = const_pool.tile([128, 512], F32)
            if not has_pick:
                nc.vector.memset(out_all[:], 0.0)
            if not has_cols:
                nc.vector.memset(colS_s[:], 0.0)

            with (tc.For_i(0, R) if R > 1 else nullcontext()):
                psC = psC_pool.tile([128, 512], F32, tag="psC")
                colS_idx = [0]
                # colS matmuls wait 2 subtiles behind dist matmuls on the PE
                # queue so a pending colS never delays the next psum refill
                pending = []
                # per-tile state for the 2-deep software pipeline
                st = {}

                def emit_pending(flush=False):
                    while pending and (flush or len(pending) > 2):
                        expo_p, chunks = pending.pop(0)
                        for blk in chunks:
                            idx = colS_idx[0]
                            nc.tensor.matmul(
                                psC[0:16, :],
                                oneh_s[:, 15 - blk:31 - blk],
                                expo_p[:, blk * 512:(blk + 1) * 512],
                                start=(idx == 0), stop=(idx == 16 * NMT - 1))
                            colS_idx[0] += 1

                def emit_dist_exp(t):
                    expo = expo_pool.tile([128, N], BF16, tag="expo")
                    if t == 0 and R == 1:
                        # ramp-special: each subtile uses one replica's chunk
                        # set so exp starts as soon as that replica's DMA
                        # lands (replicas arrive in order g0, g1, g2, g3)
                        subtiles = [(0, [0, 4, 8, 12]), (1, [1, 5, 9]),
                                    (0, [2, 6, 10, 14]), (1, [3, 7, 11]),
                                    (0, [13, 15])]
                    else:
                        pat = SUBTILES_EVEN if t % 2 == 0 else SUBTILES_ODD
                        subtiles = [(pid, list(range(off // 512,
                                                     (off + width) // 512)))
                                    for (pid, off, width) in pat]
                    for (pid, chunks) in subtiles:
                        pool = psA_pool if pid == 0 else psB_pool
                        psw = 2048 if pid == 0 else 1536
                        ps = pool.tile([128, psw], F32, tag=f"ps{pid}")
                        for k, c in enumerate(chunks):
                            gb = 32 * (c % 4)
                            nc.tensor.matmul(
                                ps[:, k * 512:(k + 1) * 512],
                                ab_s[gb:gb + 13, t * 128:(t + 1) * 128],
                                ab_s[gb:gb + 13, 2048 + (c // 4) * 512:
                                     2048 + (c // 4 + 1) * 512],
                                start=True, stop=True, tile_position=(gb, 0))
                        if has_cols:
                            emit_pending()
                        if has_exp:
                            w = 512 * len(chunks)
                            step = (chunks[1] - chunks[0]
                                    if len(chunks) > 1 else 1)
                            if step == 1:
                                ov = expo[:, chunks[0] * 512:
                                          chunks[0] * 512 + w]
                            else:
                                # arithmetic chunk sequence: strided view
                                c0 = chunks[0] // step
                                s0 = chunks[0] % step
                                ov = expo[:].rearrange(
                                    "p (c s x) -> p c s x", s=step, x=512
                                )[:, c0:c0 + len(chunks), s0:s0 + 1, :]
                            nc.scalar.activation(ov, ps[:, 0:w], Exp,
                                                 scale=T_SOFT)
                        if has_cols:
                            pending.append((expo, chunks))
                    return expo

                def emit_tree_piece(t, expo, scores, c0, c1):
                    # per-chunk (64-pt) maxes for chunks [c0, c1) via tt-max tree
                    nch = c1 - c0
                    prev = expo[:, c0 * 64:c1 * 64]
                    for w in (32, 16, 8, 4, 2, 1):
                        if w == 1:
                            cur = scores[:, c0:c1]
                        else:
                            lvl = tree_pool.tile([128, 128 * w], BF16,
                                                 name=f"lvl{w}", tag=f"L{w}")
                            cur = lvl[:, 0:nch * w]
                        v = prev.rearrange("p (c x) -> p c x", x=2 * w)
                        nc.vector.tensor_tensor(
                            out=cur.rearrange("p (c x) -> p c x", x=w),
                            in0=v[:, :, 0:w], in1=v[:, :, w:2 * w], op=MAX)
                        prev = cur

                def emit_pick(t, scores):
                    top8 = small_pool.tile([128, 8], BF16, tag="top8")
                    ci = small_pool.tile([128, 8], U32, tag="ci")
                    nc.vector.max(top8[:], scores[:])
                    nc.vector.max_index(ci[:], top8[:], scores[:])
                    wnd = small_pool.tile([128, 256], F16, tag="wnd")
                    nc.gpsimd.indirect_dma_start(
                        out=wnd[:], out_offset=None, in_=w_d[:],
                        in_offset=bass.IndirectOffsetOnAxis(ap=ci[:, 0:1], axis=0))
                    nc.vector.tensor_copy(out_all[:, t:t + 1], top8[:, 0:1])
                    st[t] = {"ci": ci, "wnd": wnd}

                def emit_refine(t):
                    s = st[t]
                    wnd = s["wnd"]
                    a0 = asml_s[:, 4 * t + 0:4 * t + 1]
                    a1 = asml_s[:, 4 * t + 1:4 * t + 2]
                    a2 = asml_s[:, 4 * t + 2:4 * t + 3]
                    t1 = small_pool.tile([128, 64], F16, tag="t1")
                    t2 = small_pool.tile([128, 64], F16, tag="t2")
                    t3 = small_pool.tile([128, 64], F16, tag="t3")
                    nc.vector.tensor_scalar(out=t1[:], in0=wnd[:, 0:64],
                                            scalar1=a0, scalar2=None, op0=MULT)
                    nc.vector.tensor_scalar(out=t2[:], in0=wnd[:, 64:128],
                                            scalar1=a1, scalar2=None, op0=MULT)
                    nc.vector.tensor_scalar(out=t3[:], in0=wnd[:, 128:192],
                                            scalar1=a2, scalar2=None, op0=MULT)
                    s1 = small_pool.tile([128, 64], F16, tag="s1")
                    s2 = small_pool.tile([128, 64], F16, tag="s2")
                    negw = small_pool.tile([128, 64], F16, tag="negw")
                    nc.vector.tensor_tensor(out=s1[:], in0=t1[:], in1=t2[:], op=ADD)
                    nc.vector.tensor_tensor(out=s2[:], in0=t3[:], in1=wnd[:, 192:256], op=ADD)
                    nc.vector.tensor_tensor(out=negw[:], in0=s1[:], in1=s2[:], op=ADD)
                    wtop = small_pool.tile([128, 8], F16, tag="wtop")
                    wli = small_pool.tile([128, 8], U32, tag="wli")
                    nc.vector.max(wtop[:], negw[:])
                    nc.vector.max_index(wli[:], wtop[:], negw[:])
                    cf = small_pool.tile([128, 1], F32, tag="cf")
                    lf = small_pool.tile([128, 1], F32, tag="lf")
                    mf = small_pool.tile([128, 1], F32, tag="mf")
                    mi = small_pool.tile([128, 1], U32, tag="mi")
                    nc.vector.tensor_copy(cf[:], s["ci"][:, 0:1])
                    nc.vector.tensor_copy(lf[:], wli[:, 0:1])
                    nc.vector.tensor_scalar(out=mf[:], in0=cf[:], scalar1=64.0,
                                            scalar2=lf[:, 0:1], op0=MULT, op1=ADD)
                    nc.vector.tensor_copy(mi[:], mf[:])
                    g = small_pool.tile([128, DA], F16, tag="g")
                    nc.gpsimd.indirect_dma_start(
                        out=g[:], out_offset=None, in_=inattr_d[:],
                        in_offset=bass.IndirectOffsetOnAxis(ap=mi[:, 0:1], axis=0))
                    s["g"] = g

                def emit_attr(t):
                    g = st.pop(t)["g"]
                    oat = outattr_s[:, DA * t:DA * (t + 1)]
                    diffa = small_pool.tile([128, DA - 4], F32, tag="diffa")
                    diffb = small_pool.tile([128, DA - 4], F32, tag="diffb")
                    nc.vector.tensor_tensor(out=diffa[:], in0=oat[:, 4:DA],
                                            in1=g[:, 4:DA], op=SUB)
                    nc.vector.tensor_tensor(out=diffb[:], in0=g[:, 4:DA],
                                            in1=oat[:, 4:DA], op=SUB)
                    nc.vector.tensor_tensor(out=bigabs[:, 52 * t:52 * t + 52],
                                            in0=diffa[:], in1=diffb[:], op=MAX)
                    rotm = small_pool.tile([128, 4], F32, tag="rotm")
                    rotd = small_pool.tile([128, 1], F32, tag="rotd")
                    rotn = small_pool.tile([128, 1], F32, tag="rotn")
                    nc.vector.tensor_tensor(out=rotm[:], in0=oat[:, 0:4],
                                            in1=g[:, 0:4], op=MULT)
                    nc.vector.tensor_reduce(rotd[:], rotm[:], axis=AX, op=ADD)
                    nc.vector.tensor_scalar(out=rotn[:], in0=rotd[:], scalar1=-1.0,
                                            scalar2=None, op0=MULT)
                    nc.vector.tensor_tensor(out=out_all[:, 16 + t:17 + t],
                                            in0=rotd[:], in1=rotn[:], op=MAX)

                # DVE emission order per iteration is readiness-ordered so no
                # ready op FIFO-blocks behind a not-yet-ready one: tree piece 1
                # unblocks at exp(t,s4), piece 2 at exp(t,s5); refine(t-1) and
                # attr(t-2) are ready when emitted.
                def emit_attr_reduce(t0, t1):
                    # per-tile |attr diff| group sums for tiles [t0, t1)
                    bav = bigabs[:, 52 * t0:52 * t1].rearrange(
                        "p (t d) -> p t d", d=52)
                    for lo, hi, col in ((0, 3, 32), (3, 4, 48), (4, 7, 64),
                                        (7, 52, 80)):
                        nc.vector.tensor_reduce(
                            out_all[:, col + t0:col + t1], bav[:, :, lo:hi],
                            axis=AX, op=ADD)

                for t in range(NMT + 2):
                    if t < NMT:
                        expo = emit_dist_exp(t)
                        if has_tree:
                            scores = tree_pool.tile([128, 128], BF16,
                                                    tag="scores")
                            if t == NMT - 1:
                                # last tile: per-subtile subtrees so only the
                                # final 1024-col piece trails the last exp
                                subtiles = (SUBTILES_EVEN if t % 2 == 0
                                            else SUBTILES_ODD)
                                for (_, off, width) in subtiles:
                                    emit_tree_piece(t, expo, scores, off // 64,
                                                    (off + width) // 64)
                            else:
                                emit_tree_piece(t, expo, scores, 0, 112)
                                emit_tree_piece(t, expo, scores, 112, 128)
                            if has_pick:
                                emit_pick(t, scores)
                        if t == NMT - 1 and has_cols:
                            emit_pending(flush=True)
                            nc.vector.tensor_copy(colS_s[:], psC[:])
                    if not has_pick:
                        continue
                    if 1 <= t <= NMT:
                        emit_refine(t - 1)
                    if t >= 2:
                        emit_attr(t - 2)
                    if t == NMT - 1:
                        # tiles 0..13 attrs are complete once attr(13) above
                        # ran; reducing their group sums here keeps only
                        # tiles 14-15 in the drain tail
                        emit_attr_reduce(0, NMT - 2)
                    if t == NMT:
                        emit_attr_reduce(NMT - 2, NMT - 1)
                if has_pick:
                    emit_attr_reduce(NMT - 1, NMT)

            nc.sync.dma_start(out_d[:], out_all[:])
            nc.sync.dma_start(outcs_d[:], colS_s[:])

    nc.compile()
    return nc


def _build_runner():
    """Build the jitted SPMD callable ONCE (jax retrace per call is ~130ms)."""
    import jax
    from jax.sharding import Mesh, PartitionSpec
    from jax.experimental.shard_map import shard_map
    from concourse import mybir
    import concourse.bass2jax as b2j

    nc = _build_program()
    b2j.install_neuronx_cc_hook()

    partition_name = nc.partition_id_tensor.name if nc.partition_id_tensor else None
    in_names, out_names, out_avals, zero_outs = [], [], [], []
    for alloc in nc.m.functions[0].allocations:
        if not isinstance(alloc, mybir.MemoryLocationSet):
            continue
        name = alloc.memorylocations[0].name
        if alloc.kind == "ExternalInput":
            if name != partition_name:
                in_names.append(name)
        elif alloc.kind == "ExternalOutput":
            out_names.append(name)
            shape = tuple(alloc.tensor_shape)
            dtype = mybir.dt.np(alloc.dtype)
            out_avals.append(jax.core.ShapedArray(shape, dtype))
            zero_outs.append(np.zeros(shape, dtype))
    n_params = len(in_names)
    n_outs = len(out_avals)
    all_in_names = list(in_names) + list(out_names)
    if partition_name is not None:
        all_in_names.append(partition_name)
    donate = tuple(range(n_params, n_params + n_outs))

    def _body(*args):
        operands = list(args)
        if partition_name is not None:
            operands.append(b2j.partition_id_tensor())
        outs = b2j._bass_exec_p.bind(
            *operands,
            out_avals=tuple(out_avals),
            in_names=tuple(all_in_names),
            out_names=tuple(out_names),
            lowering_input_output_aliases=(),
            sim_require_finite=True,
            sim_require_nnan=True,
            nc=nc,
        )
        return tuple(outs)

    devices = jax.devices()[:NCORES]
    mesh = Mesh(np.asarray(devices), ("core",))
    in_specs = (PartitionSpec("core"),) * (n_params + n_outs)
    out_specs = (PartitionSpec("core"),) * n_outs
    sharded = jax.jit(
        shard_map(_body, mesh=mesh, in_specs=in_specs, out_specs=out_specs,
                  check_rep=False),
        donate_argnums=donate, keep_unused=True,
    )

    from jax.sharding import NamedSharding
    core_sharding = NamedSharding(mesh, PartitionSpec("core"))

    def prepare(in_maps):
        concat_in = [
            np.concatenate([np.asarray(in_maps[c][name]) for c in range(NCORES)], axis=0)
            for name in in_names
        ]
        return [jax.device_put(a, core_sharding) for a in concat_in]

    def execute(dev_in):
        concat_zeros = [np.zeros((NCORES * z.shape[0], *z.shape[1:]), z.dtype)
                        for z in zero_outs]
        out_arrs = sharded(*dev_in, *concat_zeros)
        return [
            {name: np.asarray(out_arrs[i]).reshape(NCORES, *out_avals[i].shape)[c]
             for i, name in enumerate(out_names)}
            for c in range(NCORES)
        ]

    def run(in_maps):
        return execute(prepare(in_maps))

    run.prepare = prepare
    run.execute = execute
    return run


def _prep_core_inputs(core, in_xyz, in_attr_cat, out_xyz, out_attr_cat):
    import ml_dtypes

    b = core // SHARDS
    s = core % SHARDS
    a_xyz = np.ascontiguousarray(out_xyz[b, s * MLOC:(s + 1) * MLOC]).astype(np.float32)
    b_xyz = np.ascontiguousarray(in_xyz[b]).astype(np.float32)

    twoa = (2.0 * a_xyz.astype(np.float64)).astype(np.float32)
    ah = _rn12(twoa)
    al = _rn12(twoa - ah)
    bb = b_xyz.astype(np.float32)
    bh = _rn12(bb)
    bl = _rn12(bb - bh)
    na = -(a_xyz.astype(np.float64) ** 2).sum(-1)
    nb = -(b_xyz.astype(np.float64) ** 2).sum(-1)
    nah = _rn12(na.astype(np.float32))
    nal = _rn12((na - nah.astype(np.float64)).astype(np.float32))
    nbh = _rn12(nb.astype(np.float32))
    nbl = _rn12((nb - nbh.astype(np.float64)).astype(np.float32))
    om = np.ones((MLOC,), np.float32)
    on = np.ones((N,), np.float32)
    A13 = np.stack([ah[:, 0], ah[:, 0], al[:, 0],
                    ah[:, 1], ah[:, 1], al[:, 1],
                    ah[:, 2], ah[:, 2], al[:, 2],
                    nah, nal, om, om], axis=0)
    B13 = np.stack([bh[:, 0], bl[:, 0], bh[:, 0],
                    bh[:, 1], bl[:, 1], bh[:, 1],
                    bh[:, 2], bl[:, 2], bh[:, 2],
                    on, on, nbh, nbl], axis=0)
    ab = np.empty((13, 4 * 4096), np.float32)
    for g in range(4):
        ab[:, g * 4096:g * 4096 + 2048] = A13
        for k in range(4):
            c = 4 * k + g
            ab[:, g * 4096 + 2048 + k * 512:g * 4096 + 2048 + (k + 1) * 512] = \
                B13[:, c * 512:(c + 1) * 512]

    W = np.empty((128, 256), np.float16)
    nbf = nb.astype(np.float32)
    for c in range(128):
        sl = slice(c * 64, (c + 1) * 64)
        W[c, 0:64] = 2.0 * b_xyz[sl, 0]
        W[c, 64:128] = 2.0 * b_xyz[sl, 1]
        W[c, 128:192] = 2.0 * b_xyz[sl, 2]
        W[c, 192:256] = nbf[sl]

    naf = na.astype(np.float32)
    asml = np.stack([a_xyz[:, 0], a_xyz[:, 1], a_xyz[:, 2], naf], axis=1)
    asml_tiled = np.ascontiguousarray(
        asml.reshape(NMT, 128, 4).transpose(1, 0, 2).reshape(128, NMT * 4))

    oa = out_attr_cat[b, s * MLOC:(s + 1) * MLOC]
    oa_tiled = np.ascontiguousarray(
        oa.reshape(NMT, 128, DA).transpose(1, 0, 2).reshape(128, NMT * DA))

    oneh = np.zeros((128, 32), ml_dtypes.bfloat16)
    oneh[:, 15] = 1.0

    return {
        "ab": ab,
        "w": W,
        "asml": asml_tiled,
        "in_attr": np.ascontiguousarray(in_attr_cat[b]),
        "out_attr": oa_tiled,
        "oneh": oneh,
    }


def kernel(in_xyz, in_rot, in_scale, in_opacity, in_sh_dc, in_sh_rest,
           out_xyz, out_rot, out_scale, out_opacity, out_sh_dc, out_sh_rest):
    if "run" not in _cache:
        _cache["run"] = _build_runner()
    run = _cache["run"]

    in_attr_cat = np.concatenate(
        [in_rot, in_scale, in_opacity, in_sh_dc, in_sh_rest], axis=2
    ).astype(np.float16)
    out_attr_cat = np.concatenate(
        [out_rot, out_scale, out_opacity, out_sh_dc, out_sh_rest], axis=2
    ).astype(np.float16)

    in_maps = [
        _prep_core_inputs(c, in_xyz, in_attr_cat, out_xyz, out_attr_cat)
        for c in range(NCORES)
    ]
    # Retry once: a crashed prior tenant can leave a core transiently wedged
    # (NRT_EXEC_UNIT_UNRECOVERABLE); it recovers after one failed attempt.
    outs_all = None
    last_err = None
    for _attempt in range(3):
        try:
            outs_all = run(in_maps)
            break
        except Exception as e:  # noqa: BLE001
            last_err = e
            import time as _time
            _time.sleep(3.0)
    if outs_all is None:
        raise last_err

    row_sum = rot_sum = scale_sum = opac_sum = shdc_sum = shrest_sum = col_sum = 0.0
    for b in range(B):
        S = np.zeros((16, 512), np.float64)
        for s in range(SHARDS):
            o = outs_all[b * SHARDS + s]["out_all"]
            S += outs_all[b * SHARDS + s]["out_cs"][0:16].astype(np.float64)
            score = o[:, 0:16].astype(np.float64)
            d2 = np.where(score > 0.0,
                          -np.log(np.maximum(score, 1e-300)) / T_SOFT, CLAMP_D2)
            row_sum += np.sqrt(np.clip(d2, 0.0, None)).sum()
            rot_sum += o[:, 16:32].sum()
            scale_sum += o[:, 32:48].sum()
            opac_sum += o[:, 48:64].sum()
            shdc_sum += o[:, 64:80].sum()
            shrest_sum += o[:, 80:96].sum()
        d2c = np.where(S > 0.0, -np.log(np.maximum(S, 1e-300)) / T_SOFT, CLAMP_D2)
        col_sum += np.sqrt(np.clip(d2c, 0.0, None)).sum()

    BM = B * M
    BN = B * N
    pos_loss = (row_sum / BM + col_sum / BN) / 2.0
    rot_loss = 1.0 - rot_sum / BM
    scale_loss = scale_sum / (BM * 3)
    opacity_loss = opac_sum / BM
    sh_loss = shdc_sum / (BM * 3) + shrest_sum / (BM * 45)
    total = (POS_W * pos_loss + ROT_W * rot_loss + SCALE_W * scale_loss
             + OPAC_W * opacity_loss + SH_W * sh_loss)
    return np.float32(total)
